# revision 7
# baseline (speedup 1.0000x reference)
"""GQA attention kernel for 8 TRN2 NeuronCores.

Problem: B=2, T=2048, D=2048, H=16 q-heads, KV=4 kv-heads, HD=128, RoPE,
non-causal softmax, out projection. f32 reference.

Sharding: 8 cores = 2 batches x 4 kv-groups. Core c handles batch c//4 and
kv-group c%4 (4 q heads + 1 kv head). Each core computes a partial output
x @ wq_g -> attention -> (heads g) @ wo_g^T: full [T, D] partial summed on
host over the 4 groups of each batch (tensor-parallel unshard).

On-device layout: everything transposed ([hd, t], hd=128=partition dim).
All matmul operands are bf16 (fp32 PSUM accumulate): bf16 enables the PE's
fast-weight-load path and halves DMA + DVE traffic. Measured rel err of the
all-bf16 pipeline vs the fp32 reference is ~1e-2 (threshold 2e-2).

Even a fully-overlapped LDWEIGHTS steals SBUF->PE streaming bandwidth from
the moving operand (~43ns per 512-col matmul, measured), so stationary
operands are reused across consecutive matmuls where possible and a
post-compile pass drops the redundant InstLdweights that tile_legalize
emits per matmul. Each HWDGE queue delivers ~108 GB/s and the gpsimd SWDGE
has a ~9us cold start, so the startup schedule interleaves weight chunks
with x chunks in exact consumption order:
 - K+V projections run as one pass per t-tile (g-chunk-major, K and V
   interleaved) sized so x DMA stays ahead of PE consumption.
 - Q projections run g-chunk-major over tt-pairs (one weight-chunk load
   feeds two t-tiles); all pair-(0,1) passes run before any pair-(2,3)
   pass so the second half of x has ~60us to arrive. The (2,3) K pass is
   hoisted between them and the (2,3) V pass runs last: the final phase-1
   PSUM tile then frees via a fast scalar evac instead of a 3.3us RoPE
   chain, and phase 2's score tiles allocate from the *same* PSUM pool, so
   there is no pool-boundary barrier into phase 2.
 - out-projection pieces run head-major: one otn chunk load feeds two
   512-wide output column tiles.
 - scores computed transposed: ST[s, t] = k^T q per s-chunk; softmax over s
   (partitions) uses exp on ACT + bf16 chunk-adds on DVE + a ones-matmul
   partition-reduce-broadcast on PE; normalization folded into the OT evac.
 - phase 2 is one flat software pipeline over all 16 (tt, head) pairs: the
   ST stream runs one s-chunk ahead of PV across head boundaries, with
   softmax epilogues and out-projection pieces drained as PE filler (a few
   pieces held back to cover the final head's softmax-epilogue latency;
   tail pieces alternate between two PSUM pools so evacuation latency never
   blocks the next piece).
"""
import os
import sys

for _p in ("/opt/trn_rl_repo", "/root/.axon_site/_ro/trn_rl_repo"):
    if os.path.isdir(_p) and _p not in sys.path:
        sys.path.append(_p)

import numpy as np
import ml_dtypes

import concourse.bass as bass
import concourse.tile as tile
from concourse.tile import add_dep_helper
from concourse import bacc, mybir
from concourse import bass_utils
from concourse.bass_utils import run_bass_kernel_spmd

# If a caller enables tracing (BASS_TRACE=1), artifact upload may have no
# bucket access in this container; fall back to the local dir.
_orig_upload = bass_utils.upload_artifacts


def _safe_upload(tmpdir):
    try:
        return _orig_upload(tmpdir)
    except Exception:
        return tmpdir


bass_utils.upload_artifacts = _safe_upload

B, T, D = 2, 2048, 2048
H, KV, HD = 16, 4, 128
NR = H // KV  # 4 q heads per kv group
NCORES = 8
ROPE_BASE = 10000.0
SCALE = float(HD) ** -0.5

F32R = mybir.dt.float32r
F32 = mybir.dt.float32
BF16 = mybir.dt.bfloat16

_cache = {}


def _elide_redundant_ldweights(nc):
    """Drop InstLdweights that reload the weights already resident in the PE
    array (same weights AP as the previous load, no semaphore sync of its
    own). tile_legalize emits one load per matmul; the PE keeps the
    stationary operand across matmuls, so consecutive same-weight matmuls
    only need the first load (validated on hardware)."""
    removed = 0
    for f in nc.m.functions:
        for b in f.blocks:
            insts = b.instructions
            keep, last_key = [], None
            for ins in insts:
                t = type(ins).__name__
                if t == "InstLdweights":
                    key = (str(ins.ins[0]), bool(ins.is_transpose),
                           ins.perf_mode)
                    if key == last_key and ins.sync_info is None:
                        removed += 1
                        continue
                    last_key = key
                elif t == "InstDrain":
                    last_key = None
                keep.append(ins)
            if len(keep) != len(insts):
                insts[:] = keep
    return removed


def _build_nc():
    nc = bacc.Bacc("TRN2", target_bir_lowering=False, debug=False,
                   num_devices=NCORES)

    xt_e = nc.dram_tensor("xt", [128, 16, T], BF16, kind="ExternalInput").ap()
    wqt_e = [nc.dram_tensor(f"wqt{j}", [128, 16, HD], BF16,
                            kind="ExternalInput").ap() for j in range(NR)]
    wkt_e = nc.dram_tensor("wkt", [128, 16, HD], BF16, kind="ExternalInput").ap()
    wvt_e = nc.dram_tensor("wvt", [128, 16, HD], BF16, kind="ExternalInput").ap()
    wot_e = nc.dram_tensor("wot", [128, NR, D], BF16, kind="ExternalInput").ap()
    cos_e = nc.dram_tensor("cosa", [128, T], F32R, kind="ExternalInput").ap()
    sin_e = nc.dram_tensor("sina", [128, T], F32R, kind="ExternalInput").ap()
    ident_e = nc.dram_tensor("ident", [128, 128], BF16, kind="ExternalInput").ap()
    ones_e = nc.dram_tensor("ones", [128, 128], BF16, kind="ExternalInput").ap()
    out_e = nc.dram_tensor("out", [T, D], BF16, kind="ExternalOutput").ap()

    with tile.TileContext(nc) as tc:
        import contextlib
        with contextlib.ExitStack() as ctx:
            consts = ctx.enter_context(tc.tile_pool(name="consts", bufs=1))
            weights = ctx.enter_context(tc.tile_pool(name="weights", bufs=1))
            acts = ctx.enter_context(tc.tile_pool(name="acts", bufs=1))

            cos_sb = consts.tile([128, T], F32R, tag="cos")
            sin_sb = consts.tile([128, T], F32R, tag="sin")
            ident_sb = consts.tile([128, 128], BF16, tag="ident")
            ones_sb = consts.tile([128, 128], BF16, tag="ones")
            scratch_sb = consts.tile([128, 2], F32R, tag="scratch")
            wkt_sb = weights.tile([128, 16, HD], BF16, tag="wkt")
            wvt_sb = weights.tile([128, 16, HD], BF16, tag="wvt")
            wqt_sb = [weights.tile([128, 16, HD], BF16, tag=f"wqt{j}",
                                   name=f"wqt{j}_sb") for j in range(NR)]
            wot_sb = weights.tile([128, NR, D], BF16, tag="wot")
            x_sb = acts.tile([128, 16, T], BF16, tag="xsb")  # full x, resident

            # --- startup DMA schedule (order == consumption order) ----------
            # sync HWDGE: wkt chunks interleaved with even-g x chunks.
            # scalar HWDGE: ident, wvt chunks interleaved with odd-g x
            #   chunks, cos/sin lower-half quarters slotted before their
            #   RoPE consumers.
            # gpsimd SWDGE (~9us cold start): cos/sin upper half first
            #   (small, needed by the tt2/tt3 RoPEs), then wq heads in
            #   consumption order, ones, wot.
            nc.scalar.dma_start(out=ident_sb, in_=ident_e)
            # trigger the ACT exp table load (~2.7us) during phase 1 so the
            # first real exp doesn't pay it
            nc.scalar.activation(scratch_sb, ident_sb[:, 0:2],
                                 mybir.ActivationFunctionType.Exp, scale=1.0)
            # Full-row transfers (4KB per-partition lines) — small-line
            # transfers measurably throttle the HWDGE queues.
            nc.sync.dma_start(out=wkt_sb, in_=wkt_e)
            nc.scalar.dma_start(out=wvt_sb, in_=wvt_e)
            for g in range(16):
                q = nc.sync if g % 2 == 0 else nc.scalar
                q.dma_start(out=x_sb[:, g, :], in_=xt_e[:, g, :])
            # gpsimd SWDGE (fast after its ~9us cold start), consumption order
            nc.gpsimd.dma_start(out=cos_sb[:, 0:1024], in_=cos_e[:, 0:1024])
            nc.gpsimd.dma_start(out=sin_sb[:, 0:1024], in_=sin_e[:, 0:1024])
            for j in range(NR):
                nc.gpsimd.dma_start(out=wqt_sb[j], in_=wqt_e[j])
            nc.gpsimd.dma_start(out=cos_sb[:, 1024:], in_=cos_e[:, 1024:])
            nc.gpsimd.dma_start(out=sin_sb[:, 1024:], in_=sin_e[:, 1024:])
            nc.gpsimd.dma_start(out=ones_sb, in_=ones_e)
            nc.gpsimd.dma_start(out=wot_sb, in_=wot_e)

            qtr = [acts.tile([128, T], BF16, tag=f"qtr{j}", name=f"qtr{j}") for j in range(NR)]
            ktr = acts.tile([128, T], BF16, tag="ktr")
            v_sb = acts.tile([128, 16, HD], BF16, tag="vsb")  # v natural, s-chunked

            # p1ps carries the projection PSUM tiles in phase 1 AND the
            # score tiles in phase 2 (same shape/tag), so there is no
            # pool-boundary barrier between the phases.
            with tc.tile_pool(name="p1ps", bufs=2, space="PSUM") as p1ps:
                # ---------------- Phase 1: projections + RoPE + vT ---------
                with tc.tile_pool(name="rope", bufs=2) as rope_pool, \
                     tc.tile_pool(name="rotps", bufs=2, space="PSUM") as rotps:

                    def rope(src, dst, tsl):
                        # dst = src*cos + rotate_half(src)*sin on DVE via
                        # partition-shifted PSUM reads (lower-half sign
                        # folded into the host sin table); add on gpsimd.
                        t1 = rope_pool.tile([128, 512], BF16, tag="t1", name="t1")
                        nc.vector.tensor_mul(t1, src, cos_sb[:, tsl])
                        t2 = rope_pool.tile([128, 512], BF16, tag="t2", name="t2")
                        nc.vector.tensor_mul(t2[0:64, :], src[64:128, :],
                                             sin_sb[0:64, tsl])
                        nc.vector.tensor_mul(t2[64:128, :], src[0:64, :],
                                             sin_sb[64:128, tsl])
                        nc.gpsimd.tensor_add(dst, t1, t2)

                    def v_evac(vslice, tt):
                        # copy vT psum -> sbuf bf16, PE-transpose 128-blocks
                        vt_sb = rope_pool.tile([128, 512], BF16, tag="vt",
                                               name=f"vt_{tt}")
                        nc.scalar.copy(vt_sb, vslice)
                        for vb in range(4):
                            tr_ps = rotps.tile([128, 128], BF16, tag="rot")
                            nc.tensor.transpose(
                                tr_ps, vt_sb[:, vb * 128:(vb + 1) * 128],
                                ident_sb)
                            nc.vector.tensor_copy(v_sb[:, tt * 4 + vb, :], tr_ps)

                    # K+V pass per t-tile for tt 0..2 (paced to x delivery)
                    for tt in range(3):
                        tsl = slice(tt * 512, (tt + 1) * 512)
                        kv = p1ps.tile([128, 2, 512], F32, tag="p1",
                                       name=f"kv_{tt}")
                        for g in range(16):
                            nc.tensor.matmul(kv[:, 0, :], wkt_sb[:, g, :],
                                             x_sb[:, g, tsl],
                                             start=(g == 0), stop=(g == 15))
                            nc.tensor.matmul(kv[:, 1, :], wvt_sb[:, g, :],
                                             x_sb[:, g, tsl],
                                             start=(g == 0), stop=(g == 15))
                        rope(kv[:, 0, :], ktr[:, tsl], tsl)
                        v_evac(kv[:, 1, :], tt)

                    def q_pass(j, pair):
                        qps = p1ps.tile([128, 2, 512], F32, tag="p1",
                                        name=f"qps_{j}_{pair[0]}")
                        for g in range(16):
                            for ti, tt in enumerate(pair):
                                tsl = slice(tt * 512, (tt + 1) * 512)
                                nc.tensor.matmul(qps[:, ti, :],
                                                 wqt_sb[j][:, g, :],
                                                 x_sb[:, g, tsl],
                                                 start=(g == 0), stop=(g == 15))
                        for ti, tt in enumerate(pair):
                            tsl = slice(tt * 512, (tt + 1) * 512)
                            rope(qps[:, ti, :], qtr[j][:, tsl], tsl)

                    for j in range(NR):
                        q_pass(j, (0, 1))
                    # tt3 K pass here so its RoPE drains during the (2,3)
                    # Q passes
                    tsl3 = slice(3 * 512, 4 * 512)
                    k3 = p1ps.tile([128, 2, 512], F32, tag="p1", name="k3")
                    for g in range(16):
                        nc.tensor.matmul(k3[:, 0, :], wkt_sb[:, g, :],
                                         x_sb[:, g, tsl3],
                                         start=(g == 0), stop=(g == 15))
                    rope(k3[:, 0, :], ktr[:, tsl3], tsl3)
                    for j in range(NR):
                        q_pass(j, (2, 3))
                    # tt3 V pass last: its PSUM tile frees via the fast
                    # scalar evac, so phase 2's second score tile never
                    # waits on a RoPE chain
                    v3 = p1ps.tile([128, 2, 512], F32, tag="p1", name="v3")
                    for g in range(16):
                        nc.tensor.matmul(v3[:, 1, :], wvt_sb[:, g, :],
                                         x_sb[:, g, tsl3],
                                         start=(g == 0), stop=(g == 15))
                    v_evac(v3[:, 1, :], 3)

                # ------------- Phase 2+3: attention + out projection -------
                # One flat software pipeline over 16 heads x 8 s-steps: the
                # ST stream runs one step ahead of PV across head
                # boundaries. Softmax epilogues and out-projection pieces
                # drain as PE filler.
                with tc.tile_pool(name="p2sb", bufs=3) as p2sb, \
                     tc.tile_pool(name="dens", bufs=3) as dens, \
                     tc.tile_pool(name="otn", bufs=2) as otnp, \
                     tc.tile_pool(name="ostg", bufs=4) as ostg, \
                     tc.tile_pool(name="outps", bufs=2, space="PSUM") as outps, \
                     tc.tile_pool(name="otps", bufs=2, space="PSUM") as otps:
                    HEADS = [(tt, h) for tt in range(4) for h in range(NR)]
                    NG = len(HEADS) * 8  # 128 global pipeline steps
                    ex_tiles = {}
                    den_tiles = {}
                    ot_tiles = {}
                    otn_tiles = {}
                    stage_tiles = {}
                    epi_q = []    # deferred softmax epilogues
                    piece_q = []  # deferred out-projection pieces

                    def out_piece(tt, tkc, otn_t, dts, pool):
                        # half a t-chunk of the out projection, head-major so
                        # the otn chunk load feeds both column tiles; results
                        # stage into a [128, 4, 512] tile so the out DMA is
                        # one full-row transfer (4KB per-partition lines)
                        rows = slice(tt * 512 + tkc * 128,
                                     tt * 512 + (tkc + 1) * 128)
                        if dts[0] == 0:
                            stage_tiles[(tt, tkc)] = ostg.tile(
                                [128, 4, 512], BF16, tag="ostg",
                                name=f"o_sb_{tt}_{tkc}")
                        o_st = stage_tiles[(tt, tkc)]
                        o_ps = {dt: pool.tile([128, 512], F32,
                                              tag="ops" if pool is outps else "ot",
                                              name=f"o_ps_{tt}_{tkc}_{dt}")
                                for dt in dts}
                        for hh in range(NR):
                            for dt in dts:
                                nc.tensor.matmul(
                                    o_ps[dt],
                                    otn_t[:, hh, tkc * 128:(tkc + 1) * 128],
                                    wot_sb[:, hh, dt * 512:(dt + 1) * 512],
                                    start=(hh == 0), stop=(hh == NR - 1))
                        for dt in dts:
                            if dt % 2 == 0:
                                nc.vector.tensor_copy(o_st[:, dt, :], o_ps[dt])
                            else:
                                nc.scalar.copy(o_st[:, dt, :], o_ps[dt])
                        if dts[0] == 2:
                            del stage_tiles[(tt, tkc)]
                            q = nc.sync if (tt * 4 + tkc) % 2 == 0 else nc.scalar
                            q.dma_start(out=out_e[rows, :], in_=o_st)

                    def issue_st(g):
                        hi, s = divmod(g, 8)
                        tt, h = HEADS[hi]
                        tsl = slice(tt * 512, (tt + 1) * 512)
                        st = p1ps.tile([128, 2, 512], F32, tag="p1",
                                       name=f"st_{g}")
                        for half in range(2):
                            sc = s * 2 + half
                            nc.tensor.matmul(st[:, half, :],
                                             ktr[:, sc * 128:(sc + 1) * 128],
                                             qtr[h][:, tsl],
                                             start=True, stop=True)
                        ex = p2sb.tile([128, 2, 512], BF16, tag="exp",
                                       name=f"ex_{g}")
                        nc.scalar.activation(ex, st,
                                             mybir.ActivationFunctionType.Exp,
                                             scale=SCALE)
                        ex_tiles[g] = ex
                        if s == 0:
                            den = dens.tile([128, 2, 512], BF16, tag="den",
                                            name=f"den_{hi}")
                            den_tiles[hi] = den
                            nc.vector.tensor_copy(den, ex)
                        else:
                            den = den_tiles[hi]
                            nc.vector.tensor_add(den, den, ex)

                    def issue_pv(g):
                        hi, s = divmod(g, 8)
                        if s == 0:
                            ot_tiles[hi] = otps.tile([128, 512], F32,
                                                     tag="ot", name=f"ot_{hi}")
                        ot = ot_tiles[hi]
                        ex = ex_tiles.pop(g)
                        for half in range(2):
                            sc = s * 2 + half
                            nc.tensor.matmul(ot, v_sb[:, sc, :],
                                             ex[:, half, :],
                                             start=(sc == 0), stop=(sc == 15))
                        if s == 7:
                            epi_q.append(hi)

                    def epilogue(hi):
                        tt, h = HEADS[hi]
                        den = den_tiles.pop(hi)
                        ot = ot_tiles.pop(hi)
                        otn_t = otn_tiles[tt]
                        # partition-reduce+broadcast denominator on PE, both
                        # halves into one PSUM bank (one ones load)
                        bc_ps = outps.tile([128, 512], F32, tag="ops",
                                           name=f"bc_{hi}")
                        nc.tensor.matmul(bc_ps, ones_sb, den[:, 0, :],
                                         start=True, stop=False)
                        nc.tensor.matmul(bc_ps, ones_sb, den[:, 1, :],
                                         start=False, stop=True)
                        rden = dens.tile([128, 512], F32, tag="rden",
                                         name=f"rden_{hi}")
                        nc.vector.reciprocal_approx_fast(rden, bc_ps)
                        nc.vector.tensor_tensor(out=otn_t[:, h, :], in0=ot,
                                                in1=rden,
                                                op=mybir.AluOpType.mult)
                        if h == NR - 1:
                            for tkc in range(4):
                                for dts in ((0, 1), (2, 3)):
                                    piece_q.append((tt, tkc, otn_t, dts))

                    for g in range(NG + 1):
                        if g < NG:
                            hi, s = divmod(g, 8)
                            tt, h = HEADS[hi]
                            if s == 0 and h == 0:
                                otn_tiles[tt] = otnp.tile([128, NR, 512],
                                                          BF16, tag="otn",
                                                          name=f"otn_{tt}")
                            issue_st(g)
                        if g >= 1:
                            issue_pv(g - 1)
                            s1 = (g - 1) % 8
                            if s1 == 1 and epi_q:
                                epilogue(epi_q.pop(0))
                            # hold a few pieces back to fill the final
                            # head's epilogue-latency bubble at stream end
                            if s1 in (3, 5) and len(piece_q) > 3:
                                out_piece(*piece_q.pop(0), outps)
                    # tail: last epilogue + remaining out projection;
                    # alternate PSUM pools so evac latency never blocks PE
                    while epi_q:
                        epilogue(epi_q.pop(0))
                    for i in range(len(piece_q)):
                        out_piece(*piece_q[i], otps if i % 2 else outps)
                    piece_q.clear()
    nc.compile()
    n = _elide_redundant_ldweights(nc)
    sys.stderr.write(f"kernel: elided {n} redundant LDWEIGHTS\n")
    return nc


def _get_nc():
    if "nc" not in _cache:
        _cache["nc"] = _build_nc()
    return _cache["nc"]


def _host_consts():
    if "consts" in _cache:
        return _cache["consts"]
    inv = 1.0 / (ROPE_BASE ** (np.arange(0, HD, 2, dtype=np.float64) / HD))
    freqs = np.outer(np.arange(T, dtype=np.float64), inv)  # [T, 64]
    emb = np.concatenate([freqs, freqs], axis=-1)  # [T, 128]
    cos_t = np.cos(emb).T.astype(np.float32).copy()  # [128, T]
    sin_t = np.sin(emb).T.astype(np.float32).copy()
    sin_t[:64, :] *= -1.0  # rotate-half sign folded in (see rope())
    ident = np.eye(128, dtype=np.float32).astype(ml_dtypes.bfloat16)
    ones = np.ones((128, 128), dtype=ml_dtypes.bfloat16)
    _cache["consts"] = (cos_t, sin_t, ident, ones)
    return _cache["consts"]


def _in_maps(x, wq, wk, wv, wo):
    cos_t, sin_t, ident, ones = _host_consts()
    bf = ml_dtypes.bfloat16
    maps = []
    for c in range(NCORES):
        b, g = c // KV, c % KV
        xt = np.ascontiguousarray(
            x[b].reshape(T, 16, 128).transpose(2, 1, 0)).astype(bf)
        wq_g = wq[g * NR * HD:(g + 1) * NR * HD]  # [512, D]
        # per-head contiguous slices: wqt{j}[p, dc, jc] = wq_g[j*128+jc, dc*128+p]
        wq_h = wq_g.reshape(NR, HD, 16, 128).transpose(0, 3, 2, 1)  # [j, p, dc, jc]
        wk_g = wk[g * HD:(g + 1) * HD]
        wkt = np.ascontiguousarray(wk_g.reshape(HD, 16, 128).transpose(2, 1, 0))
        wv_g = wv[g * HD:(g + 1) * HD]
        wvt = np.ascontiguousarray(wv_g.reshape(HD, 16, 128).transpose(2, 1, 0))
        wo_g = wo[:, g * NR * HD:(g + 1) * NR * HD]  # [D, 512]
        wot = np.ascontiguousarray(
            wo_g.reshape(D, NR, 128).transpose(2, 1, 0)).astype(bf)
        m = {
            "xt": xt, "wkt": wkt.astype(bf),
            "wvt": wvt.astype(bf), "wot": wot,
            "cosa": cos_t, "sina": sin_t,
            "ident": ident, "ones": ones,
        }
        for j in range(NR):
            m[f"wqt{j}"] = np.ascontiguousarray(wq_h[j]).astype(bf)
        maps.append(m)
    return maps


def run_spmd(x, wq, wk, wv, wo, **kw):
    nc = _get_nc()
    maps = _in_maps(x, wq, wk, wv, wo)
    return run_bass_kernel_spmd(nc, maps, core_ids=list(range(NCORES)), **kw)


def kernel(x, wq, wk, wv, wo):
    res = run_spmd(x, wq, wk, wv, wo)
    out = np.zeros((B, T, D), dtype=np.float32)
    for c in range(NCORES):
        out[c // KV] += res.results[c]["out"].astype(np.float32)
    return out


# revision 11
# speedup vs baseline: 1.0074x; 1.0074x over previous
"""GQA attention kernel for 8 TRN2 NeuronCores.

Problem: B=2, T=2048, D=2048, H=16 q-heads, KV=4 kv-heads, HD=128, RoPE,
non-causal softmax, out projection. f32 reference.

Sharding: 8 cores = 2 batches x 4 kv-groups. Core c handles batch c//4 and
kv-group c%4 (4 q heads + 1 kv head). Each core computes a partial output
x @ wq_g -> attention -> (heads g) @ wo_g^T: full [T, D] partial summed on
host over the 4 groups of each batch (tensor-parallel unshard).

On-device layout: everything transposed ([hd, t], hd=128=partition dim).
All matmul operands are bf16 (fp32 PSUM accumulate): bf16 enables the PE's
fast-weight-load path and halves DMA + DVE traffic. Measured rel err of the
all-bf16 pipeline vs the fp32 reference is ~1e-2 (threshold 2e-2).

Even a fully-overlapped LDWEIGHTS steals SBUF->PE streaming bandwidth from
the moving operand (~43ns per 512-col matmul, measured), so stationary
operands are reused across consecutive matmuls where possible and a
post-compile pass drops the redundant InstLdweights that tile_legalize
emits per matmul. Each HWDGE queue delivers ~108 GB/s and the gpsimd SWDGE
has a ~9us cold start, so the startup schedule interleaves weight chunks
with x chunks in exact consumption order:
 - K+V projections run as one pass per t-tile (g-chunk-major, K and V
   interleaved) sized so x DMA stays ahead of PE consumption.
 - Q projections run g-chunk-major over tt-pairs (one weight-chunk load
   feeds two t-tiles); all pair-(0,1) passes run before any pair-(2,3)
   pass so the second half of x has ~60us to arrive. The (2,3) K pass is
   hoisted between them and the (2,3) V pass runs last: the final phase-1
   PSUM tile then frees via a fast scalar evac instead of a 3.3us RoPE
   chain, and phase 2's score tiles allocate from the *same* PSUM pool, so
   there is no pool-boundary barrier into phase 2.
 - out-projection pieces run head-major: one otn chunk load feeds two
   512-wide output column tiles.
 - scores computed transposed: ST[s, t] = k^T q per s-chunk; softmax over s
   (partitions) uses exp on ACT + bf16 chunk-adds on DVE + a ones-matmul
   partition-reduce-broadcast on PE; normalization folded into the OT evac.
 - phase 2 is one flat software pipeline over all 16 (tt, head) pairs: the
   ST stream runs one s-chunk ahead of PV across head boundaries, with
   softmax epilogues and out-projection pieces drained as PE filler (a few
   pieces held back to cover the final head's softmax-epilogue latency;
   tail pieces alternate between two PSUM pools so evacuation latency never
   blocks the next piece).
"""
import os
import sys

for _p in ("/opt/trn_rl_repo", "/root/.axon_site/_ro/trn_rl_repo"):
    if os.path.isdir(_p) and _p not in sys.path:
        sys.path.append(_p)

import numpy as np
import ml_dtypes

import concourse.bass as bass
import concourse.tile as tile
from concourse.tile import add_dep_helper
from concourse import bacc, mybir
from concourse import bass_utils
from concourse.bass_utils import run_bass_kernel_spmd

# If a caller enables tracing (BASS_TRACE=1), artifact upload may have no
# bucket access in this container; fall back to the local dir.
_orig_upload = bass_utils.upload_artifacts


def _safe_upload(tmpdir):
    try:
        return _orig_upload(tmpdir)
    except Exception:
        return tmpdir


bass_utils.upload_artifacts = _safe_upload

B, T, D = 2, 2048, 2048
H, KV, HD = 16, 4, 128
NR = H // KV  # 4 q heads per kv group
NCORES = 8
ROPE_BASE = 10000.0
SCALE = float(HD) ** -0.5

F32R = mybir.dt.float32r
F32 = mybir.dt.float32
BF16 = mybir.dt.bfloat16

_cache = {}


def _elide_redundant_ldweights(nc):
    """Drop InstLdweights that reload the weights already resident in the PE
    array (same weights AP as the previous load, no semaphore sync of its
    own). tile_legalize emits one load per matmul; the PE keeps the
    stationary operand across matmuls, so consecutive same-weight matmuls
    only need the first load (validated on hardware)."""
    removed = 0
    for f in nc.m.functions:
        for b in f.blocks:
            insts = b.instructions
            keep, last_key = [], None
            for ins in insts:
                t = type(ins).__name__
                if t == "InstLdweights":
                    key = (str(ins.ins[0]), bool(ins.is_transpose),
                           ins.perf_mode)
                    if key == last_key and ins.sync_info is None:
                        removed += 1
                        continue
                    last_key = key
                elif t == "InstDrain":
                    last_key = None
                keep.append(ins)
            if len(keep) != len(insts):
                insts[:] = keep
    return removed


def _build_nc():
    nc = bacc.Bacc("TRN2", target_bir_lowering=False, debug=False,
                   num_devices=NCORES)

    xt_e = nc.dram_tensor("xt", [128, 16, T], BF16, kind="ExternalInput").ap()
    wqt_e = [nc.dram_tensor(f"wqt{j}", [128, 16, HD], BF16,
                            kind="ExternalInput").ap() for j in range(NR)]
    wkt_e = nc.dram_tensor("wkt", [128, 16, HD], BF16, kind="ExternalInput").ap()
    wvt_e = nc.dram_tensor("wvt", [128, 16, HD], BF16, kind="ExternalInput").ap()
    wot_e = nc.dram_tensor("wot", [128, NR, D], BF16, kind="ExternalInput").ap()
    cos_e = nc.dram_tensor("cosa", [128, T], F32R, kind="ExternalInput").ap()
    sin_e = nc.dram_tensor("sina", [128, T], F32R, kind="ExternalInput").ap()
    ident_e = nc.dram_tensor("ident", [128, 128], BF16, kind="ExternalInput").ap()
    ones_e = nc.dram_tensor("ones", [128, 128], BF16, kind="ExternalInput").ap()
    out_e = nc.dram_tensor("out", [T, D], BF16, kind="ExternalOutput").ap()

    with tile.TileContext(nc) as tc:
        import contextlib
        with contextlib.ExitStack() as ctx:
            consts = ctx.enter_context(tc.tile_pool(name="consts", bufs=1))
            weights = ctx.enter_context(tc.tile_pool(name="weights", bufs=1))
            acts = ctx.enter_context(tc.tile_pool(name="acts", bufs=1))

            cos_sb = consts.tile([128, T], F32R, tag="cos")
            sin_sb = consts.tile([128, T], F32R, tag="sin")
            ident_sb = consts.tile([128, 128], BF16, tag="ident")
            ones_sb = consts.tile([128, 128], BF16, tag="ones")
            scratch_sb = consts.tile([128, 2], F32R, tag="scratch")
            wkt_sb = weights.tile([128, 16, HD], BF16, tag="wkt")
            wvt_sb = weights.tile([128, 16, HD], BF16, tag="wvt")
            wqt_sb = [weights.tile([128, 16, HD], BF16, tag=f"wqt{j}",
                                   name=f"wqt{j}_sb") for j in range(NR)]
            wot_sb = weights.tile([128, NR, D], BF16, tag="wot")
            x_sb = acts.tile([128, 16, T], BF16, tag="xsb")  # full x, resident

            # --- startup DMA schedule (order == consumption order) ----------
            # sync HWDGE: wkt chunks interleaved with even-g x chunks.
            # scalar HWDGE: ident, wvt chunks interleaved with odd-g x
            #   chunks, cos/sin lower-half quarters slotted before their
            #   RoPE consumers.
            # gpsimd SWDGE (~9us cold start): cos/sin upper half first
            #   (small, needed by the tt2/tt3 RoPEs), then wq heads in
            #   consumption order, ones, wot.
            nc.scalar.dma_start(out=ident_sb, in_=ident_e)
            # trigger the ACT exp table load (~2.7us) during phase 1 so the
            # first real exp doesn't pay it
            nc.scalar.activation(scratch_sb, ident_sb[:, 0:2],
                                 mybir.ActivationFunctionType.Exp, scale=1.0)
            # Each HWDGE queue tops out near ~105 GB/s regardless of line
            # size, so x goes out as quarter-row chunks in EXACT consumption
            # order (t-tile major, g minor, parity-striped across the two
            # HWDGE queues) with the K/V weight chunks interleaved at their
            # consumption points. Everything else rides the gpsimd SWDGE
            # (~146 GB/s after a ~9us cold start) in consumption order.
            ts0 = slice(0, 512)
            for i in range(4):
                gsl = slice(4 * i, 4 * i + 4)
                nc.sync.dma_start(out=wkt_sb[:, gsl, :], in_=wkt_e[:, gsl, :])
                nc.scalar.dma_start(out=wvt_sb[:, gsl, :], in_=wvt_e[:, gsl, :])
                for g in (4 * i, 4 * i + 2):
                    nc.sync.dma_start(out=x_sb[:, g, ts0], in_=xt_e[:, g, ts0])
                for g in (4 * i + 1, 4 * i + 3):
                    nc.scalar.dma_start(out=x_sb[:, g, ts0], in_=xt_e[:, g, ts0])
            for tt in range(1, 4):
                tsl = slice(tt * 512, (tt + 1) * 512)
                for g in range(16):
                    q = nc.sync if g % 2 == 0 else nc.scalar
                    q.dma_start(out=x_sb[:, g, tsl], in_=xt_e[:, g, tsl])
            # gpsimd SWDGE: cos/sin quarters just ahead of their RoPE
            # consumers, wq heads in consumption order, then late weights
            for qs, dst in (((0, 512), 0), ((512, 1024), 1)):
                nc.gpsimd.dma_start(out=cos_sb[:, qs[0]:qs[1]],
                                    in_=cos_e[:, qs[0]:qs[1]])
                nc.gpsimd.dma_start(out=sin_sb[:, qs[0]:qs[1]],
                                    in_=sin_e[:, qs[0]:qs[1]])
                nc.gpsimd.dma_start(out=wqt_sb[dst], in_=wqt_e[dst])
            nc.gpsimd.dma_start(out=cos_sb[:, 1024:], in_=cos_e[:, 1024:])
            nc.gpsimd.dma_start(out=sin_sb[:, 1024:], in_=sin_e[:, 1024:])
            nc.gpsimd.dma_start(out=wqt_sb[2], in_=wqt_e[2])
            nc.gpsimd.dma_start(out=wqt_sb[3], in_=wqt_e[3])
            nc.gpsimd.dma_start(out=ones_sb, in_=ones_e)
            nc.gpsimd.dma_start(out=wot_sb, in_=wot_e)

            qtr = [acts.tile([128, T], BF16, tag=f"qtr{j}", name=f"qtr{j}") for j in range(NR)]
            ktr = acts.tile([128, T], BF16, tag="ktr")
            v_sb = acts.tile([128, 16, HD], BF16, tag="vsb")  # v natural, s-chunked

            # p1ps carries the projection PSUM tiles in phase 1 AND the
            # score tiles in phase 2 (same shape/tag), so there is no
            # pool-boundary barrier between the phases.
            with tc.tile_pool(name="p1ps", bufs=2, space="PSUM") as p1ps:
                # ---------------- Phase 1: projections + RoPE + vT ---------
                with tc.tile_pool(name="rope", bufs=2) as rope_pool, \
                     tc.tile_pool(name="rotps", bufs=2, space="PSUM") as rotps:

                    def rope(src, dst, tsl):
                        # dst = src*cos + rotate_half(src)*sin on DVE via
                        # partition-shifted PSUM reads (lower-half sign
                        # folded into the host sin table); add on gpsimd.
                        t1 = rope_pool.tile([128, 512], BF16, tag="t1", name="t1")
                        nc.vector.tensor_mul(t1, src, cos_sb[:, tsl])
                        t2 = rope_pool.tile([128, 512], BF16, tag="t2", name="t2")
                        nc.vector.tensor_mul(t2[0:64, :], src[64:128, :],
                                             sin_sb[0:64, tsl])
                        nc.vector.tensor_mul(t2[64:128, :], src[0:64, :],
                                             sin_sb[64:128, tsl])
                        nc.gpsimd.tensor_add(dst, t1, t2)

                    def v_evac(vslice, tt):
                        # copy vT psum -> sbuf bf16, PE-transpose 128-blocks
                        vt_sb = rope_pool.tile([128, 512], BF16, tag="vt",
                                               name=f"vt_{tt}")
                        nc.scalar.copy(vt_sb, vslice)
                        for vb in range(4):
                            tr_ps = rotps.tile([128, 128], BF16, tag="rot")
                            nc.tensor.transpose(
                                tr_ps, vt_sb[:, vb * 128:(vb + 1) * 128],
                                ident_sb)
                            nc.vector.tensor_copy(v_sb[:, tt * 4 + vb, :], tr_ps)

                    # K+V pass per t-tile for tt 0..2 (paced to x delivery)
                    for tt in range(3):
                        tsl = slice(tt * 512, (tt + 1) * 512)
                        kv = p1ps.tile([128, 2, 512], F32, tag="p1",
                                       name=f"kv_{tt}")
                        for g in range(16):
                            nc.tensor.matmul(kv[:, 0, :], wkt_sb[:, g, :],
                                             x_sb[:, g, tsl],
                                             start=(g == 0), stop=(g == 15))
                            nc.tensor.matmul(kv[:, 1, :], wvt_sb[:, g, :],
                                             x_sb[:, g, tsl],
                                             start=(g == 0), stop=(g == 15))
                        rope(kv[:, 0, :], ktr[:, tsl], tsl)
                        v_evac(kv[:, 1, :], tt)

                    def q_pass(j, pair):
                        qps = p1ps.tile([128, 2, 512], F32, tag="p1",
                                        name=f"qps_{j}_{pair[0]}")
                        for g in range(16):
                            for ti, tt in enumerate(pair):
                                tsl = slice(tt * 512, (tt + 1) * 512)
                                nc.tensor.matmul(qps[:, ti, :],
                                                 wqt_sb[j][:, g, :],
                                                 x_sb[:, g, tsl],
                                                 start=(g == 0), stop=(g == 15))
                        for ti, tt in enumerate(pair):
                            tsl = slice(tt * 512, (tt + 1) * 512)
                            rope(qps[:, ti, :], qtr[j][:, tsl], tsl)

                    for j in range(NR):
                        q_pass(j, (0, 1))
                    # tt3 K pass here so its RoPE drains during the (2,3)
                    # Q passes
                    tsl3 = slice(3 * 512, 4 * 512)
                    k3 = p1ps.tile([128, 2, 512], F32, tag="p1", name="k3")
                    for g in range(16):
                        nc.tensor.matmul(k3[:, 0, :], wkt_sb[:, g, :],
                                         x_sb[:, g, tsl3],
                                         start=(g == 0), stop=(g == 15))
                    rope(k3[:, 0, :], ktr[:, tsl3], tsl3)
                    for j in range(NR):
                        q_pass(j, (2, 3))
                    # tt3 V pass last: its PSUM tile frees via the fast
                    # scalar evac, so phase 2's second score tile never
                    # waits on a RoPE chain
                    v3 = p1ps.tile([128, 2, 512], F32, tag="p1", name="v3")
                    for g in range(16):
                        nc.tensor.matmul(v3[:, 1, :], wvt_sb[:, g, :],
                                         x_sb[:, g, tsl3],
                                         start=(g == 0), stop=(g == 15))
                    v_evac(v3[:, 1, :], 3)

                # ------------- Phase 2+3: attention + out projection -------
                # One flat software pipeline over 16 heads x 8 s-steps: the
                # ST stream runs one step ahead of PV across head
                # boundaries. Softmax epilogues and out-projection pieces
                # drain as PE filler.
                with tc.tile_pool(name="p2sb", bufs=3) as p2sb, \
                     tc.tile_pool(name="dens", bufs=3) as dens, \
                     tc.tile_pool(name="otn", bufs=2) as otnp, \
                     tc.tile_pool(name="ostg", bufs=4) as ostg, \
                     tc.tile_pool(name="outps", bufs=2, space="PSUM") as outps, \
                     tc.tile_pool(name="otps", bufs=2, space="PSUM") as otps:
                    HEADS = [(tt, h) for tt in range(4) for h in range(NR)]
                    NG = len(HEADS) * 8  # 128 global pipeline steps
                    ex_tiles = {}
                    den_tiles = {}
                    ot_tiles = {}
                    otn_tiles = {}
                    stage_tiles = {}
                    epi_q = []    # deferred softmax epilogues
                    piece_q = []  # deferred out-projection pieces

                    def out_piece(tt, tkc, otn_t, dts, pool):
                        # half a t-chunk of the out projection, head-major so
                        # the otn chunk load feeds both column tiles; results
                        # stage into a [128, 4, 512] tile so the out DMA is
                        # one full-row transfer (4KB per-partition lines)
                        rows = slice(tt * 512 + tkc * 128,
                                     tt * 512 + (tkc + 1) * 128)
                        if dts[0] == 0:
                            stage_tiles[(tt, tkc)] = ostg.tile(
                                [128, 4, 512], BF16, tag="ostg",
                                name=f"o_sb_{tt}_{tkc}")
                        o_st = stage_tiles[(tt, tkc)]
                        o_ps = {dt: pool.tile([128, 512], F32,
                                              tag="ops" if pool is outps else "ot",
                                              name=f"o_ps_{tt}_{tkc}_{dt}")
                                for dt in dts}
                        for hh in range(NR):
                            for dt in dts:
                                nc.tensor.matmul(
                                    o_ps[dt],
                                    otn_t[:, hh, tkc * 128:(tkc + 1) * 128],
                                    wot_sb[:, hh, dt * 512:(dt + 1) * 512],
                                    start=(hh == 0), stop=(hh == NR - 1))
                        for dt in dts:
                            if dt % 2 == 0:
                                nc.vector.tensor_copy(o_st[:, dt, :], o_ps[dt])
                            else:
                                nc.scalar.copy(o_st[:, dt, :], o_ps[dt])
                        if dts[0] == 2:
                            del stage_tiles[(tt, tkc)]
                            q = nc.sync if (tt * 4 + tkc) % 2 == 0 else nc.scalar
                            q.dma_start(out=out_e[rows, :], in_=o_st)

                    def issue_st(g):
                        hi, s = divmod(g, 8)
                        tt, h = HEADS[hi]
                        tsl = slice(tt * 512, (tt + 1) * 512)
                        st = p1ps.tile([128, 2, 512], F32, tag="p1",
                                       name=f"st_{g}")
                        for half in range(2):
                            sc = s * 2 + half
                            nc.tensor.matmul(st[:, half, :],
                                             ktr[:, sc * 128:(sc + 1) * 128],
                                             qtr[h][:, tsl],
                                             start=True, stop=True)
                        ex = p2sb.tile([128, 2, 512], BF16, tag="exp",
                                       name=f"ex_{g}")
                        nc.scalar.activation(ex, st,
                                             mybir.ActivationFunctionType.Exp,
                                             scale=SCALE)
                        ex_tiles[g] = ex
                        if s == 0:
                            den = dens.tile([128, 2, 512], BF16, tag="den",
                                            name=f"den_{hi}")
                            den_tiles[hi] = den
                            nc.vector.tensor_copy(den, ex)
                        else:
                            den = den_tiles[hi]
                            nc.vector.tensor_add(den, den, ex)

                    def issue_pv(g):
                        hi, s = divmod(g, 8)
                        if s == 0:
                            ot_tiles[hi] = otps.tile([128, 512], F32,
                                                     tag="ot", name=f"ot_{hi}")
                        ot = ot_tiles[hi]
                        ex = ex_tiles.pop(g)
                        for half in range(2):
                            sc = s * 2 + half
                            nc.tensor.matmul(ot, v_sb[:, sc, :],
                                             ex[:, half, :],
                                             start=(sc == 0), stop=(sc == 15))
                        if s == 7:
                            epi_q.append(hi)

                    def epilogue(hi):
                        tt, h = HEADS[hi]
                        den = den_tiles.pop(hi)
                        ot = ot_tiles.pop(hi)
                        otn_t = otn_tiles[tt]
                        # partition-reduce+broadcast denominator on PE, both
                        # halves into one PSUM bank (one ones load)
                        bc_ps = outps.tile([128, 512], F32, tag="ops",
                                           name=f"bc_{hi}")
                        nc.tensor.matmul(bc_ps, ones_sb, den[:, 0, :],
                                         start=True, stop=False)
                        nc.tensor.matmul(bc_ps, ones_sb, den[:, 1, :],
                                         start=False, stop=True)
                        rden = dens.tile([128, 512], F32, tag="rden",
                                         name=f"rden_{hi}")
                        nc.vector.reciprocal_approx_fast(rden, bc_ps)
                        nc.vector.tensor_tensor(out=otn_t[:, h, :], in0=ot,
                                                in1=rden,
                                                op=mybir.AluOpType.mult)
                        if h == NR - 1:
                            for tkc in range(4):
                                for dts in ((0, 1), (2, 3)):
                                    piece_q.append((tt, tkc, otn_t, dts))

                    for g in range(NG + 1):
                        if g < NG:
                            hi, s = divmod(g, 8)
                            tt, h = HEADS[hi]
                            if s == 0 and h == 0:
                                otn_tiles[tt] = otnp.tile([128, NR, 512],
                                                          BF16, tag="otn",
                                                          name=f"otn_{tt}")
                            issue_st(g)
                        if g >= 1:
                            issue_pv(g - 1)
                            s1 = (g - 1) % 8
                            if s1 == 1 and epi_q:
                                epilogue(epi_q.pop(0))
                            # hold a few pieces back to fill the final
                            # head's epilogue-latency bubble at stream end
                            if s1 in (3, 5) and len(piece_q) > 3:
                                out_piece(*piece_q.pop(0), outps)
                    # tail: last epilogue + remaining out projection;
                    # alternate PSUM pools so evac latency never blocks PE
                    while epi_q:
                        epilogue(epi_q.pop(0))
                    for i in range(len(piece_q)):
                        out_piece(*piece_q[i], otps if i % 2 else outps)
                    piece_q.clear()
    nc.compile()
    n = _elide_redundant_ldweights(nc)
    sys.stderr.write(f"kernel: elided {n} redundant LDWEIGHTS\n")
    return nc


def _get_nc():
    if "nc" not in _cache:
        _cache["nc"] = _build_nc()
    return _cache["nc"]


def _host_consts():
    if "consts" in _cache:
        return _cache["consts"]
    inv = 1.0 / (ROPE_BASE ** (np.arange(0, HD, 2, dtype=np.float64) / HD))
    freqs = np.outer(np.arange(T, dtype=np.float64), inv)  # [T, 64]
    emb = np.concatenate([freqs, freqs], axis=-1)  # [T, 128]
    cos_t = np.cos(emb).T.astype(np.float32).copy()  # [128, T]
    sin_t = np.sin(emb).T.astype(np.float32).copy()
    sin_t[:64, :] *= -1.0  # rotate-half sign folded in (see rope())
    ident = np.eye(128, dtype=np.float32).astype(ml_dtypes.bfloat16)
    ones = np.ones((128, 128), dtype=ml_dtypes.bfloat16)
    _cache["consts"] = (cos_t, sin_t, ident, ones)
    return _cache["consts"]


def _in_maps(x, wq, wk, wv, wo):
    cos_t, sin_t, ident, ones = _host_consts()
    bf = ml_dtypes.bfloat16
    maps = []
    for c in range(NCORES):
        b, g = c // KV, c % KV
        xt = np.ascontiguousarray(
            x[b].reshape(T, 16, 128).transpose(2, 1, 0)).astype(bf)
        wq_g = wq[g * NR * HD:(g + 1) * NR * HD]  # [512, D]
        # per-head contiguous slices: wqt{j}[p, dc, jc] = wq_g[j*128+jc, dc*128+p]
        wq_h = wq_g.reshape(NR, HD, 16, 128).transpose(0, 3, 2, 1)  # [j, p, dc, jc]
        wk_g = wk[g * HD:(g + 1) * HD]
        wkt = np.ascontiguousarray(wk_g.reshape(HD, 16, 128).transpose(2, 1, 0))
        wv_g = wv[g * HD:(g + 1) * HD]
        wvt = np.ascontiguousarray(wv_g.reshape(HD, 16, 128).transpose(2, 1, 0))
        wo_g = wo[:, g * NR * HD:(g + 1) * NR * HD]  # [D, 512]
        wot = np.ascontiguousarray(
            wo_g.reshape(D, NR, 128).transpose(2, 1, 0)).astype(bf)
        m = {
            "xt": xt, "wkt": wkt.astype(bf),
            "wvt": wvt.astype(bf), "wot": wot,
            "cosa": cos_t, "sina": sin_t,
            "ident": ident, "ones": ones,
        }
        for j in range(NR):
            m[f"wqt{j}"] = np.ascontiguousarray(wq_h[j]).astype(bf)
        maps.append(m)
    return maps


def run_spmd(x, wq, wk, wv, wo, **kw):
    nc = _get_nc()
    maps = _in_maps(x, wq, wk, wv, wo)
    return run_bass_kernel_spmd(nc, maps, core_ids=list(range(NCORES)), **kw)


def kernel(x, wq, wk, wv, wo):
    res = run_spmd(x, wq, wk, wv, wo)
    out = np.zeros((B, T, D), dtype=np.float32)
    for c in range(NCORES):
        out[c // KV] += res.results[c]["out"].astype(np.float32)
    return out
